# revision 5
# baseline (speedup 1.0000x reference)
"""Block-diagonal complex matmul kernel for trn2 (8 NeuronCores).

Reference computation:
  xp = take(x, perm_idx, axis=-2).reshape(B, 2, M, S)
  y_re = xp_re @ hr1 + xp_im @ hi1   (per block a of M)
  y_im = xp_re @ hi2 + xp_im @ hr2
  out  = stack([y_re, y_im], 1).reshape(B, 2, N, R)

Sharding: block dim M=1024 split across 8 cores (128 blocks each).
Permutation gather + all layout shuffles happen host-side in numpy.

Precision budget (tolerance 2e-2): weights are fp8 e3m4 scaled by 2^4
(rel err ~1.3e-2), x is fp16 scaled by 2^-4 so the product is unscaled,
accumulation in fp32 PSUM, output fp16. Mixed fp16-stationary x
fp8-moving matmul is supported by the PE (verified on hw).

Per-core device kernel, per block a:
  psum[16, 256] = x_re[:, a].T @ [hr1[a] | hi2[a]]   (start)
                + x_im[:, a].T @ [hi1[a] | hr2[a]]   (stop)
  -> cols 0:128 = y_re[a], cols 128:256 = y_im[a]
"""

import os
import numpy as np

B = 16
N = 4096
R = 32
M = 1024   # blocks
S = 128    # block size (contract dim)
NCORES = 8
MLOC = M // NCORES   # 128 blocks per core
WC = 4 * S           # 512 fp8 weight cols per block: [hr1|hi2|hi1|hr2]
XC = 2 * B           # 32 fp16 x cols per block: [re(16) | im(16)]
# blocks per DMA group: small first/last groups shrink pipeline fill/drain
GROUPS = [4, 4] + [8] * 14 + [4, 4]
assert sum(GROUPS) == MLOC
WSCALE = 16.0        # weights scaled up 2^4 into e3m4 range, x scaled down

_NC_CACHE = {}


def _build_nc():
    import concourse.bacc as bacc
    import concourse.bass as bass
    import concourse.mybir as mybir
    from concourse import tile

    x_dt = mybir.dt.float16
    w_dt = mybir.dt.float8e3
    nc = bacc.Bacc(None, target_bir_lowering=False)

    # x: contraction dim S on partitions, per block [re(16) | im(16)] cols
    x2 = nc.dram_tensor("x2", [S, MLOC * XC], x_dt, kind="ExternalInput")
    # weights: per block 512 fp8 cols = [W1 | W2], W1 = [hr1 | hi2],
    # W2 = [hi1 | hr2]
    w = nc.dram_tensor("w", [S, MLOC * WC], w_dt, kind="ExternalInput")
    y = nc.dram_tensor("y", [B, MLOC * 2 * S], x_dt, kind="ExternalOutput")

    with tile.TileContext(nc) as tc:
        with (
            tc.tile_pool(name="xp", bufs=8) as xpool,
            tc.tile_pool(name="wp", bufs=8) as wpool,
            tc.tile_pool(name="op", bufs=6) as opool,
            tc.tile_pool(name="ps", bufs=8, space=bass.MemorySpace.PSUM) as ps,
        ):
            a0 = 0
            pair_idx = 0
            for nb in GROUPS:
                wt = wpool.tile([S, nb * WC], w_dt)
                nc.sync.dma_start(wt[:], w[:, a0 * WC:(a0 + nb) * WC])
                xt = xpool.tile([S, nb * XC], x_dt)
                nc.gpsimd.dma_start(xt[:], x2[:, a0 * XC:(a0 + nb) * XC])
                ot = opool.tile([B, nb * 2 * S], x_dt)
                # two blocks share one PSUM bank; one [16, 512] copy per pair
                for p in range(nb // 2):
                    pt = ps.tile([B, 4 * S], mybir.dt.float32)
                    for h in range(2):
                        i = 2 * p + h
                        c0 = i * WC
                        w1 = wt[:, c0:c0 + 2 * S]
                        w2 = wt[:, c0 + 2 * S:c0 + 4 * S]
                        xr = xt[:, i * XC:i * XC + B]
                        xi = xt[:, i * XC + B:(i + 1) * XC]
                        psl = pt[:, h * 2 * S:(h + 1) * 2 * S]
                        nc.tensor.matmul(psl, xr, w1, start=True, stop=False)
                        nc.tensor.matmul(psl, xi, w2, start=False, stop=True)
                    osl = ot[:, p * 4 * S:(p + 1) * 4 * S]
                    if pair_idx % 2 == 0:
                        nc.scalar.copy(osl, pt[:])
                    else:
                        nc.vector.tensor_copy(osl, pt[:])
                    pair_idx += 1
                nc.sync.dma_start(y[:, a0 * 2 * S:(a0 + nb) * 2 * S], ot[:])
                a0 += nb
    nc.compile()
    return nc


def kernel(x, hr1, hi1, hr2, hi2, perm_idx):
    import ml_dtypes
    from concourse.bass_utils import run_bass_kernel_spmd

    if "nc" not in _NC_CACHE:
        _NC_CACHE["nc"] = _build_nc()
    nc = _NC_CACHE["nc"]

    x = np.asarray(x, dtype=np.float32)
    perm_idx = np.asarray(perm_idx)
    # host-side permutation gather + regroup into M blocks of size S
    xp = x[:, :, perm_idx, :].reshape(B, 2, M, S)
    # [B, 2, M, S] -> [S, M, 2, B], scaled by 1/WSCALE (exact exponent shift)
    xall = np.ascontiguousarray(
        np.transpose(xp, (3, 2, 1, 0)) * np.float32(1.0 / WSCALE)
    ).astype(np.float16)

    in_maps = []
    for c in range(NCORES):
        a0 = c * MLOC
        sl = slice(a0, a0 + MLOC)
        x2 = np.ascontiguousarray(xall[:, sl]).reshape(S, MLOC * XC)
        # per block 512 cols: [hr1 | hi2 | hi1 | hr2], scaled into e3m4 range
        wc = (np.concatenate([hr1[sl], hi2[sl], hi1[sl], hr2[sl]], axis=2)
              * np.float32(WSCALE)).astype(ml_dtypes.float8_e3m4)  # [MLOC, S, 4S]
        wc = np.ascontiguousarray(np.transpose(wc, (1, 0, 2))).reshape(S, MLOC * WC)
        in_maps.append({"x2": x2, "w": wc})

    trace = bool(os.environ.get("KERNEL_TRACE"))
    kwargs = {}
    if trace:
        kwargs["tmpdir"] = os.environ.get("KERNEL_TRACE_DIR") or None
    res = run_bass_kernel_spmd(nc, in_maps, core_ids=list(range(NCORES)), trace=trace, **kwargs)
    if trace and res.exec_time_ns is not None:
        print(f"HW exec time: {res.exec_time_ns} ns")
        _NC_CACHE["exec_time_ns"] = res.exec_time_ns
        _NC_CACHE["profile"] = res

    out = np.empty((B, 2, M, S), dtype=np.float32)
    for c in range(NCORES):
        a0 = c * MLOC
        yc = res.results[c]["y"].reshape(B, MLOC, 2, S)
        out[:, 0, a0:a0 + MLOC, :] = yc[:, :, 0, :]
        out[:, 1, a0:a0 + MLOC, :] = yc[:, :, 1, :]
    return out.reshape(B, 2, N, R)


# revision 7
# speedup vs baseline: 1.3715x; 1.3715x over previous
"""Block-diagonal complex matmul kernel for trn2 (8 NeuronCores).

Reference computation:
  xp = take(x, perm_idx, axis=-2).reshape(B, 2, M, S)
  y_re = xp_re @ hr1 + xp_im @ hi1   (per block a of M)
  y_im = xp_re @ hi2 + xp_im @ hr2
  out  = stack([y_re, y_im], 1).reshape(B, 2, N, R)

Sharding: block dim M=1024 split across 8 cores (128 blocks each).
Permutation gather + all layout shuffles happen host-side in numpy.

Precision budget (tolerance 2e-2): weights are fp8 e3m4 scaled by 2^4
(rel err ~1.3e-2), x is fp16 scaled by 2^-4 so the product is unscaled,
accumulation in fp32 PSUM, output fp16. Mixed fp16-stationary x
fp8-moving matmul is supported by the PE (verified on hw).

Per-core device kernel, per block a:
  psum[16, 256] = x_re[:, a].T @ [hr1[a] | hi2[a]]   (start)
                + x_im[:, a].T @ [hi1[a] | hr2[a]]   (stop)
  -> cols 0:128 = y_re[a], cols 128:256 = y_im[a]
"""

import os
import numpy as np

B = 16
N = 4096
R = 32
M = 1024   # blocks
S = 128    # block size (contract dim)
NCORES = 8
MLOC = M // NCORES   # 128 blocks per core
WC = 4 * S           # 512 fp8 weight cols per block: [hr1|hi2|hi1|hr2]
XC = 2 * B           # 32 fp16 x cols per block: [re(16) | im(16)]
# blocks per DMA group: small first/last groups shrink pipeline fill/drain
GROUPS = [4, 4] + [8] * 14 + [4, 4]
assert sum(GROUPS) == MLOC
WSCALE = 16.0        # weights scaled up 2^4 into e3m4 range, x scaled down

_NC_CACHE = {}


def _build_nc():
    import concourse.bacc as bacc
    import concourse.bass as bass
    import concourse.mybir as mybir
    from concourse import tile

    x_dt = mybir.dt.float16
    w_dt = mybir.dt.float8e3
    nc = bacc.Bacc(None, target_bir_lowering=False)

    # x: contraction dim S on partitions, per block [re(16) | im(16)] cols
    x2 = nc.dram_tensor("x2", [S, MLOC * XC], x_dt, kind="ExternalInput")
    # weights: per block 512 fp8 cols = [W1 | W2], W1 = [hr1 | hi2],
    # W2 = [hi1 | hr2]
    w = nc.dram_tensor("w", [S, MLOC * WC], w_dt, kind="ExternalInput")
    y = nc.dram_tensor("y", [B, MLOC * 2 * S], x_dt, kind="ExternalOutput")

    with tile.TileContext(nc) as tc:
        with (
            tc.tile_pool(name="xp", bufs=1) as xpool,
            tc.tile_pool(name="wp", bufs=8) as wpool,
            tc.tile_pool(name="op", bufs=6) as opool,
            tc.tile_pool(name="ps", bufs=8, space=bass.MemorySpace.PSUM) as ps,
        ):
            # all of x lands up front via one DMA; w groups stream behind it.
            # out-DMAs issue from the engine that produced the copies so the
            # sync queue stays a pure w-prefetch queue (no head-of-line waits).
            xt = xpool.tile([S, MLOC * XC], x_dt, name="xt")
            nc.gpsimd.dma_start(xt[:], x2[:])
            a0 = 0
            for gi, nb in enumerate(GROUPS):
                wt = wpool.tile([S, nb * WC], w_dt)
                nc.sync.dma_start(wt[:], w[:, a0 * WC:(a0 + nb) * WC])
                # out-DMA dispatch: scalar after its own copies; gpsimd (idle)
                # for vector-copy groups since DVE can't initiate DMAs
                eng = nc.scalar if gi % 2 == 0 else nc.gpsimd
                ot = opool.tile([B, nb * 2 * S], x_dt)
                # two blocks share one PSUM bank; one [16, 512] copy per pair
                for p in range(nb // 2):
                    pt = ps.tile([B, 4 * S], mybir.dt.float32)
                    for h in range(2):
                        i = 2 * p + h
                        c0 = i * WC
                        w1 = wt[:, c0:c0 + 2 * S]
                        w2 = wt[:, c0 + 2 * S:c0 + 4 * S]
                        a = a0 + i
                        xr = xt[:, a * XC:a * XC + B]
                        xi = xt[:, a * XC + B:(a + 1) * XC]
                        psl = pt[:, h * 2 * S:(h + 1) * 2 * S]
                        nc.tensor.matmul(psl, xr, w1, start=True, stop=False)
                        nc.tensor.matmul(psl, xi, w2, start=False, stop=True)
                    osl = ot[:, p * 4 * S:(p + 1) * 4 * S]
                    if gi % 2 == 0:
                        nc.scalar.copy(osl, pt[:])
                    else:
                        nc.vector.tensor_copy(osl, pt[:])
                eng.dma_start(y[:, a0 * 2 * S:(a0 + nb) * 2 * S], ot[:])
                a0 += nb
    nc.compile()
    return nc


def kernel(x, hr1, hi1, hr2, hi2, perm_idx):
    import ml_dtypes
    from concourse.bass_utils import run_bass_kernel_spmd

    if "nc" not in _NC_CACHE:
        _NC_CACHE["nc"] = _build_nc()
    nc = _NC_CACHE["nc"]

    x = np.asarray(x, dtype=np.float32)
    perm_idx = np.asarray(perm_idx)
    # host-side permutation gather + regroup into M blocks of size S
    xp = x[:, :, perm_idx, :].reshape(B, 2, M, S)
    # [B, 2, M, S] -> [S, M, 2, B], scaled by 1/WSCALE (exact exponent shift)
    xall = np.ascontiguousarray(
        np.transpose(xp, (3, 2, 1, 0)) * np.float32(1.0 / WSCALE)
    ).astype(np.float16)

    in_maps = []
    for c in range(NCORES):
        a0 = c * MLOC
        sl = slice(a0, a0 + MLOC)
        x2 = np.ascontiguousarray(xall[:, sl]).reshape(S, MLOC * XC)
        # per block 512 cols: [hr1 | hi2 | hi1 | hr2], scaled into e3m4 range
        wc = (np.concatenate([hr1[sl], hi2[sl], hi1[sl], hr2[sl]], axis=2)
              * np.float32(WSCALE)).astype(ml_dtypes.float8_e3m4)  # [MLOC, S, 4S]
        wc = np.ascontiguousarray(np.transpose(wc, (1, 0, 2))).reshape(S, MLOC * WC)
        in_maps.append({"x2": x2, "w": wc})

    trace = bool(os.environ.get("KERNEL_TRACE"))
    kwargs = {}
    if trace:
        kwargs["tmpdir"] = os.environ.get("KERNEL_TRACE_DIR") or None
    res = run_bass_kernel_spmd(nc, in_maps, core_ids=list(range(NCORES)), trace=trace, **kwargs)
    if trace and res.exec_time_ns is not None:
        print(f"HW exec time: {res.exec_time_ns} ns")
        _NC_CACHE["exec_time_ns"] = res.exec_time_ns
        _NC_CACHE["profile"] = res

    out = np.empty((B, 2, M, S), dtype=np.float32)
    for c in range(NCORES):
        a0 = c * MLOC
        yc = res.results[c]["y"].reshape(B, MLOC, 2, S)
        out[:, 0, a0:a0 + MLOC, :] = yc[:, :, 0, :]
        out[:, 1, a0:a0 + MLOC, :] = yc[:, :, 1, :]
    return out.reshape(B, 2, N, R)
